# revision 1
# baseline (speedup 1.0000x reference)
"""Trainium2 Bass kernel for ClasswiseECE (nn_ClasswiseECE).

Math: the reference collapses to
    answer = (1/(C*N)) * sum_{c,b} | conf_sum[c,b] - acc_sum[c,b] |
where, with p = softmax(logits) and s = bin(p) = trunc(15*p) (== ceil(15p)-1
except on a measure-zero boundary set),
    conf_sum[c,b] = sum_r p[r,c]   * 1{s[r,c]==b}
    acc_sum[c,b]  = sum_r 1{label_r==c} * 1{bin(p[r,label_r])==b}
(the cnt>0 guard and the cnt/N factors cancel exactly).

Device strategy (data-parallel over rows, 8 cores):
  - rows on partitions, classes on free axis; per group of B=16 128-row tiles:
    ACT computes exp, DVE computes row sums / reciprocal / p / bin index s
    (fp32->int16 truncation) / one-hot(label); 14 fused scalar_tensor_tensor
    instructions build bf16 masked tiles (s==b)*p in 2x DVE mode.
  - the otherwise-idle PE reduces over rows: ones-vector matmuls accumulate
    per-(class,bin) conf sums and per-class full sums of p in PSUM; a small
    one-hot(bin(p_label)) x one-hot(label) matmul accumulates the label
    histogram acc_sum[15,100].
  - host sums the 8 cores' 1500-element accumulators in fp64 and finishes.
Bin 14 is derived on host as dfull - sum_{b<14} conf[b] (mask identity).
Pad rows (to reach 63488 rows/core) use logits [40,-40,...], label 1: their
contributions land in known slots and are subtracted exactly on host.
"""
import numpy as np
import ml_dtypes

import concourse.bass as bass
import concourse.bacc as bacc
import concourse.mybir as mybir
from concourse.tile import TileContext
from concourse import bass_utils

F32 = mybir.dt.float32
BF16 = mybir.dt.bfloat16
I16 = mybir.dt.int16

N, C, NB = 500000, 100, 15
NCORES = 8
NMASK = 14                  # bins 0..13 via masks; bin 14 derived on host
B = 16                      # 128-row tiles per group
G = 31                      # groups per core
RPC = G * B * 128           # rows per core incl. padding = 63488
REAL_PER_CORE = N // NCORES  # 62500
PAD_PER_CORE = RPC - REAL_PER_CORE  # 988
SEGS = [(0, 5), (5, 10), (10, 14)]  # conf matmul bin segments (PSUM bank <=512)

LAST_RESULT = None  # BassKernelResults of the most recent run (for test.py)


def _build_nc() -> bass.Bass:
    nc = bacc.Bacc("TRN2", target_bir_lowering=False)

    x = nc.dram_tensor("logits", [RPC, C], F32, kind="ExternalInput")
    lab = nc.dram_tensor("labels_bf16", [RPC], BF16, kind="ExternalInput")
    xl = nc.dram_tensor("xlabel", [RPC], F32, kind="ExternalInput")
    iota_c = nc.dram_tensor("iota_c", [128, C], BF16, kind="ExternalInput")
    iota_b = nc.dram_tensor("iota_b", [128, NB], BF16, kind="ExternalInput")
    ones_in = nc.dram_tensor("ones_col", [128, 1], BF16, kind="ExternalInput")

    conf_o = nc.dram_tensor("conf", [1, NMASK * C], F32, kind="ExternalOutput")
    acc_o = nc.dram_tensor("acc", [NB, C], F32, kind="ExternalOutput")
    dfull_o = nc.dram_tensor("dfull", [1, C], F32, kind="ExternalOutput")

    # row r = (g*128 + p)*B + j: each partition holds B contiguous DRAM rows
    x_r = x.ap().rearrange("(g p j) c -> g p j c", g=G, j=B, p=128)
    lab_r = lab.ap().rearrange("(g p j) -> g p j", g=G, j=B, p=128)
    xl_r = xl.ap().rearrange("(g p j) -> g p j", g=G, j=B, p=128)

    with TileContext(nc) as tc:
        with (
            tc.tile_pool(name="const", bufs=1) as cpool,
            tc.tile_pool(name="work", bufs=2) as pool,
            tc.tile_pool(name="masks", bufs=2) as mpool,
            tc.tile_pool(name="psum", bufs=1, space="PSUM") as ppool,
            tc.tile_pool(name="outs", bufs=1) as opool,
        ):
            iota_c_t = cpool.tile([128, C], BF16)
            nc.sync.dma_start(iota_c_t[:], iota_c.ap())
            iota_b_t = cpool.tile([128, NB], BF16)
            nc.sync.dma_start(iota_b_t[:], iota_b.ap())
            ones_t = cpool.tile([128, 1], BF16)
            nc.sync.dma_start(ones_t[:], ones_in.ap())

            conf_ps = [
                ppool.tile([1, (b1 - b0) * C], F32, name=f"conf_ps{i}",
                           tag=f"conf_ps{i}")
                for i, (b0, b1) in enumerate(SEGS)
            ]
            acc_ps = ppool.tile([NB, C], F32)
            dfull_ps = ppool.tile([1, C], F32)

            for g in range(G):
                first = g == 0
                last = g == G - 1

                xt = pool.tile([128, B, C], F32, tag="xt")
                nc.sync.dma_start(xt[:], x_r[g])
                labt = pool.tile([128, B], BF16, tag="labt")
                nc.sync.dma_start(labt[:], lab_r[g])
                xlt = pool.tile([128, B], F32, tag="xlt")
                nc.sync.dma_start(xlt[:], xl_r[g])

                e = pool.tile([128, B, C], F32, tag="e")
                nc.scalar.activation(e[:], xt[:],
                                     mybir.ActivationFunctionType.Exp)
                S = pool.tile([128, B], F32, tag="S")
                nc.vector.tensor_reduce(S[:], e[:], axis=mybir.AxisListType.X,
                                        op=mybir.AluOpType.add)
                r = pool.tile([128, B], F32, tag="r")
                nc.vector.reciprocal(r[:], S[:])
                p = pool.tile([128, B, C], F32, tag="p")
                r_b = r[:].unsqueeze(2).broadcast_to([128, B, C])
                nc.vector.tensor_tensor(p[:], e[:], r_b,
                                        op=mybir.AluOpType.mult)

                s16 = pool.tile([128, B, C], I16, tag="s16")
                # HW fp32->int convert rounds to nearest: RTN(15p-0.5) ==
                # trunc(15p) away from measure-zero tie points.
                nc.vector.tensor_scalar(s16[:], p[:], 15.0, -0.5,
                                        op0=mybir.AluOpType.mult,
                                        op1=mybir.AluOpType.add)
                sbf = pool.tile([128, B, C], BF16, tag="sbf")
                nc.vector.tensor_copy(sbf[:], s16[:])
                pbf = pool.tile([128, B, C], BF16, tag="pbf")
                nc.vector.tensor_copy(pbf[:], p[:])

                oh = pool.tile([128, B, C], BF16, tag="oh")
                lab_b = labt[:].unsqueeze(2).broadcast_to([128, B, C])
                iota_c_b = iota_c_t[:].unsqueeze(1).broadcast_to([128, B, C])
                nc.vector.tensor_tensor(oh[:], lab_b, iota_c_b,
                                        op=mybir.AluOpType.is_equal)

                # label-probability path (tiny [128, B] tensors)
                el = pool.tile([128, B], F32, tag="el")
                nc.scalar.activation(el[:], xlt[:],
                                     mybir.ActivationFunctionType.Exp)
                pl = pool.tile([128, B], F32, tag="pl")
                nc.vector.tensor_tensor(pl[:], el[:], r[:],
                                        op=mybir.AluOpType.mult)
                sl16 = pool.tile([128, B], I16, tag="sl16")
                nc.vector.tensor_scalar(sl16[:], pl[:], 15.0, -0.5,
                                        op0=mybir.AluOpType.mult,
                                        op1=mybir.AluOpType.add)
                slbf = pool.tile([128, B], BF16, tag="slbf")
                nc.vector.tensor_copy(slbf[:], sl16[:])
                bhl = pool.tile([128, B, NB], BF16, tag="bhl")
                sl_b = slbf[:].unsqueeze(2).broadcast_to([128, B, NB])
                iota_b_b = iota_b_t[:].unsqueeze(1).broadcast_to([128, B, NB])
                nc.vector.tensor_tensor(bhl[:], sl_b, iota_b_b,
                                        op=mybir.AluOpType.is_equal)

                masks = mpool.tile([128, NMASK, B, C], BF16, tag="masks")
                for b in range(NMASK):
                    nc.vector.scalar_tensor_tensor(
                        masks[:, b], sbf[:], float(b), pbf[:],
                        op0=mybir.AluOpType.is_equal,
                        op1=mybir.AluOpType.mult)

                for j in range(B):
                    st = first and j == 0
                    sp = last and j == B - 1
                    for i, (b0, b1) in enumerate(SEGS):
                        nc.tensor.matmul(conf_ps[i][:], ones_t[:],
                                         masks[:, b0:b1, j, :],
                                         start=st, stop=sp)
                    nc.tensor.matmul(dfull_ps[:], ones_t[:], pbf[:, j, :],
                                     start=st, stop=sp)
                    nc.tensor.matmul(acc_ps[:], bhl[:, j, :], oh[:, j, :],
                                     start=st, stop=sp)

            conf_sb = opool.tile([1, NMASK * C], F32)
            for i, (b0, b1) in enumerate(SEGS):
                nc.vector.tensor_copy(conf_sb[:, b0 * C:b1 * C], conf_ps[i][:])
            nc.sync.dma_start(conf_o.ap(), conf_sb[:])
            acc_sb = opool.tile([NB, C], F32)
            nc.vector.tensor_copy(acc_sb[:], acc_ps[:])
            nc.sync.dma_start(acc_o.ap(), acc_sb[:])
            dfull_sb = opool.tile([1, C], F32)
            nc.vector.tensor_copy(dfull_sb[:], dfull_ps[:])
            nc.sync.dma_start(dfull_o.ap(), dfull_sb[:])
    nc.finalize()
    return nc


def _shard_inputs(logits: np.ndarray, labels: np.ndarray):
    labels_i = np.asarray(labels).astype(np.int64)
    logits = np.ascontiguousarray(np.asarray(logits, dtype=np.float32))
    xlabel = np.take_along_axis(logits, labels_i[:, None], axis=1)[:, 0]

    pad_logits = np.full((PAD_PER_CORE, C), -40.0, np.float32)
    pad_logits[:, 0] = 40.0
    pad_labels = np.full((PAD_PER_CORE,), 1, np.int64)
    pad_xlabel = np.full((PAD_PER_CORE,), -40.0, np.float32)

    iota_c_np = np.broadcast_to(
        np.arange(C, dtype=np.float32).astype(ml_dtypes.bfloat16), (128, C))
    iota_b_np = np.broadcast_to(
        np.arange(NB, dtype=np.float32).astype(ml_dtypes.bfloat16), (128, NB))
    ones_np = np.ones((128, 1), ml_dtypes.bfloat16)

    in_maps = []
    for k in range(NCORES):
        lo, hi = k * REAL_PER_CORE, (k + 1) * REAL_PER_CORE
        lg = np.concatenate([logits[lo:hi], pad_logits], axis=0)
        lb = np.concatenate([labels_i[lo:hi], pad_labels])
        xb = np.concatenate([xlabel[lo:hi], pad_xlabel])
        in_maps.append({
            "logits": np.ascontiguousarray(lg),
            "labels_bf16": lb.astype(np.float32).astype(ml_dtypes.bfloat16),
            "xlabel": np.ascontiguousarray(xb),
            "iota_c": np.ascontiguousarray(iota_c_np),
            "iota_b": np.ascontiguousarray(iota_b_np),
            "ones_col": ones_np,
        })
    return in_maps


def _finalize(results) -> np.float32:
    conf = np.zeros((NMASK, C), np.float64)
    acc = np.zeros((NB, C), np.float64)
    dfull = np.zeros((C,), np.float64)
    for res in results:
        conf += res["conf"].reshape(NMASK, C).astype(np.float64)
        acc += res["acc"].astype(np.float64)
        dfull += res["dfull"].reshape(C).astype(np.float64)

    total_pad = PAD_PER_CORE * NCORES
    dfull[0] -= total_pad       # pad rows: p[class0] == 1.0 exactly in bf16
    acc[0, 1] -= total_pad      # pad rows: label 1, p_label in bin 0

    conf14 = dfull - conf.sum(axis=0)
    d = np.concatenate([conf - acc[:NMASK], (conf14 - acc[NB - 1])[None]], 0)
    ans = np.abs(d).sum() / (C * N)
    return np.float32(ans)


def _run_pjrt(nc: bass.Bass, in_maps, time_iters: int = 0):
    """Mirror of bass2jax.run_bass_via_pjrt with pre-staged device inputs.

    Timing: the axon tunnel between this client and the NeuronCores has a
    ~70 ms round-trip latency that is pure transport, not device time, so a
    single blocking call measures the network, not the hardware. We instead
    measure steady-state throughput the way GPU kernels are benched: enqueue
    ``time_iters`` executions back-to-back, block once at the end, and
    divide. NEFF executions on the same core serialize, so per-iteration
    time is an upper bound on true device execution time.
    """
    import time
    import jax
    import concourse.mybir as _mb
    from jax.sharding import Mesh, PartitionSpec
    from jax.experimental.shard_map import shard_map
    from concourse.bass2jax import (
        install_neuronx_cc_hook, _bass_exec_p, partition_id_tensor)

    install_neuronx_cc_hook()
    partition_name = (nc.partition_id_tensor.name
                      if nc.partition_id_tensor else None)
    in_names, out_names, out_avals, zero_outs = [], [], [], []
    for alloc in nc.m.functions[0].allocations:
        if not isinstance(alloc, _mb.MemoryLocationSet):
            continue
        name = alloc.memorylocations[0].name
        if alloc.kind == "ExternalInput":
            if name != partition_name:
                in_names.append(name)
        elif alloc.kind == "ExternalOutput":
            shape = tuple(alloc.tensor_shape)
            dtype = _mb.dt.np(alloc.dtype)
            out_names.append(name)
            out_avals.append(jax.core.ShapedArray(shape, dtype))
            zero_outs.append(np.zeros(shape, dtype))
    n_params = len(in_names)
    n_outs = len(out_avals)
    all_names = in_names + out_names
    if partition_name is not None:
        all_names = all_names + [partition_name]

    def _body(*args):
        operands = list(args)
        if partition_name is not None:
            operands.append(partition_id_tensor())
        outs = _bass_exec_p.bind(
            *operands,
            out_avals=tuple(out_avals),
            in_names=tuple(all_names),
            out_names=tuple(out_names),
            lowering_input_output_aliases=(),
            sim_require_finite=True,
            sim_require_nnan=True,
            nc=nc,
        )
        return tuple(outs)

    devices = jax.devices()[:NCORES]
    mesh = Mesh(np.asarray(devices), ("core",))
    # No donation: every output element is fully written by the kernel's
    # final DMAs, so the custom call's fresh (uninitialized) result buffers
    # are fine and the zero operands can be reused across all iterations.
    sharded = jax.jit(
        shard_map(_body, mesh=mesh,
                  in_specs=(PartitionSpec("core"),) * (n_params + n_outs),
                  out_specs=(PartitionSpec("core"),) * n_outs,
                  check_rep=False),
        keep_unused=True)

    sh = jax.sharding.NamedSharding(mesh, PartitionSpec("core"))
    concat_in = [
        jax.device_put(
            np.concatenate([np.asarray(in_maps[c][nm]) for c in range(NCORES)],
                           axis=0), sh)
        for nm in in_names
    ]
    zs = [
        jax.device_put(
            np.zeros((NCORES * z.shape[0], *z.shape[1:]), z.dtype), sh)
        for z in zero_outs
    ]
    jax.block_until_ready(zs)

    out_arrs = sharded(*concat_in, *zs)
    jax.block_until_ready(out_arrs)

    times = []
    if time_iters:
        for _ in range(3):  # timed rounds; min is reported
            t0 = time.perf_counter()
            outs = [sharded(*concat_in, *zs) for _ in range(time_iters)]
            jax.block_until_ready(outs)
            times.append((time.perf_counter() - t0) / time_iters)

    results = [
        {nm: np.asarray(out_arrs[i]).reshape(NCORES, *out_avals[i].shape)[c]
         for i, nm in enumerate(out_names)}
        for c in range(NCORES)
    ]
    return results, times


def kernel(logits: np.ndarray, labels: np.ndarray) -> np.ndarray:
    global LAST_RESULT
    import os
    in_maps = _shard_inputs(logits, labels)
    nc = _build_nc()
    time_iters = int(os.environ.get("KERNEL_TIME_ITERS", "0"))
    results, times = _run_pjrt(nc, in_maps, time_iters=time_iters)
    LAST_RESULT = {"results": results, "times": times}
    return np.asarray(_finalize(results), dtype=np.float32)



# revision 2
# speedup vs baseline: 1.1134x; 1.1134x over previous
"""Trainium2 Bass kernel for ClasswiseECE (nn_ClasswiseECE).

Math: the reference collapses to
    answer = (1/(C*N)) * sum_{c,b} | conf_sum[c,b] - acc_sum[c,b] |
with p = softmax(logits); conf_sum[c,b] = sum_r p[r,c] * 1{bin(p[r,c])==b},
acc_sum[c,b] = #{r: label_r==c, bin(p[r,label_r])==b} (cnt/N factors cancel).

Bin b <=> p in (b/15, (b+1)/15], so with cumulative sums
    cum[c,b] = sum_r p[r,c] * 1{p[r,c] > b/15}        (b = 1..14)
we get conf_sum[c,b] = cum[c,b] - cum[c,b+1] (cum[c,0] = dfull = sum_r p,
cum[c,15] = 0).  The device computes cum via PE matmuls:
    out[m, (b,c)] = sum_r pbf[r,m] * step_b[r,c],   step_b = 1{pbf > b/15}
whose DIAGONAL m==c is cum[c,b]; a ones-column gives dfull.  step masks are
tensor_scalar is_gt ops (4x DVE mode on bf16), so DVE does half the work of
the scalar_tensor_tensor masked-tile approach.  Per-row softmax sums come
free from the ScalarE activation accum_out; pbf is built per row-tile j
either on ACT (exp(x - lnS) with per-partition bias) or on DVE
(tensor_scalar e*r with per-partition scalar) to balance engine load.
acc_sum accumulates as matmul(lhsT=onehot_label (host-built), rhs=
onehot_bin(p_label)).  Host sums the 8 cores' outputs in fp64, extracts the
diagonal, differences adjacent bins, and subtracts pad-row contributions.

Sharding: data-parallel over rows, 8 cores, 62500 real rows + 988 pad rows
per core.  Pad rows use logits [40,-40,...], label 1: pbf = [1,0,...] and
p_label ~ 0 land in known slots, subtracted exactly on host.
"""
import numpy as np
import ml_dtypes

import concourse.bass as bass
import concourse.bacc as bacc
import concourse.mybir as mybir
from concourse.tile import TileContext
from concourse import bass_utils

F32 = mybir.dt.float32
BF16 = mybir.dt.bfloat16
I16 = mybir.dt.int16

N, C, NB = 500000, 100, 15
NCORES = 8
NTH = 14                     # thresholds b=1..14
B = 8                        # 128-row tiles per group
G = 62                       # groups per core
RPC = G * B * 128            # rows per core incl. padding = 63488
REAL_PER_CORE = N // NCORES  # 62500
PAD_PER_CORE = RPC - REAL_PER_CORE  # 988
SEGS = [(0, 5), (5, 10), (10, 14)]  # threshold index segments (PSUM bank <=512)
ACT_J = (0, 2, 4, 6)         # row-tiles whose pbf is built on ScalarE

OUT_W = NTH * C + 1 + NB     # 1416: cum segs | dfull | acc

LAST_RESULT = None  # BassKernelResults of the most recent run (for test.py)


def _build_nc() -> bass.Bass:
    nc = bacc.Bacc("TRN2", target_bir_lowering=False)

    x = nc.dram_tensor("logits", [RPC, C], F32, kind="ExternalInput")
    xl = nc.dram_tensor("xlabel", [RPC], F32, kind="ExternalInput")
    oh = nc.dram_tensor("onehot", [RPC, C], BF16, kind="ExternalInput")
    iota_b = nc.dram_tensor("iota_b", [128, NB], BF16, kind="ExternalInput")
    ones_in = nc.dram_tensor("ones_col", [128, 1], BF16, kind="ExternalInput")

    big_o = nc.dram_tensor("big", [C, OUT_W], F32, kind="ExternalOutput")

    # row r = (g*128 + p)*B + j: each partition holds B contiguous DRAM rows
    x_r = x.ap().rearrange("(g p j) c -> g p j c", g=G, j=B, p=128)
    xl_r = xl.ap().rearrange("(g p j) -> g p j", g=G, j=B, p=128)
    oh_r = oh.ap().rearrange("(g p j) c -> g p j c", g=G, j=B, p=128)

    Exp = mybir.ActivationFunctionType.Exp
    Ln = mybir.ActivationFunctionType.Ln

    with TileContext(nc) as tc:
        with (
            tc.tile_pool(name="const", bufs=1) as cpool,
            tc.tile_pool(name="work", bufs=2) as pool,
            tc.tile_pool(name="steps", bufs=2) as spool,
            tc.tile_pool(name="psum", bufs=1, space="PSUM") as ppool,
            tc.tile_pool(name="outs", bufs=1) as opool,
        ):
            iota_b_t = cpool.tile([128, NB], BF16)
            nc.sync.dma_start(iota_b_t[:], iota_b.ap())
            ones_t = cpool.tile([128, 1], BF16)
            nc.sync.dma_start(ones_t[:], ones_in.ap())

            cum_ps = [
                ppool.tile([C, (b1 - b0) * C], F32, name=f"cum_ps{i}",
                           tag=f"cum_ps{i}")
                for i, (b0, b1) in enumerate(SEGS)
            ]
            dfull_ps = ppool.tile([C, 1], F32)
            acc_ps = ppool.tile([C, NB], F32)

            for g in range(G):
                first = g == 0
                last = g == G - 1

                xt = pool.tile([128, B, C], F32, tag="xt")
                nc.sync.dma_start(xt[:], x_r[g])
                xlt = pool.tile([128, B], F32, tag="xlt")
                nc.sync.dma_start(xlt[:], xl_r[g])
                oht = pool.tile([128, B, C], BF16, tag="oht")
                nc.sync.dma_start(oht[:], oh_r[g])

                # e = exp(x) with fused per-row sums S (ScalarE accum_out)
                et = pool.tile([128, B, C], F32, tag="et")
                St = pool.tile([128, B], F32, tag="St")
                for j in range(B):
                    nc.scalar.activation(et[:, j], xt[:, j], Exp,
                                         accum_out=St[:, j:j + 1])
                rt = pool.tile([128, B], F32, tag="rt")
                nc.vector.reciprocal(rt[:], St[:])
                mlnS = pool.tile([128, B], F32, tag="mlnS")
                nc.scalar.activation(mlnS[:], rt[:], Ln)

                # pbf = bf16(p): split per row-tile between ACT and DVE
                pbf = pool.tile([128, B, C], BF16, tag="pbf")
                for j in range(B):
                    if j in ACT_J:
                        nc.scalar.activation(pbf[:, j], xt[:, j], Exp,
                                             bias=mlnS[:, j:j + 1])
                    else:
                        nc.vector.tensor_scalar(pbf[:, j], et[:, j],
                                                rt[:, j:j + 1], None,
                                                op0=mybir.AluOpType.mult)

                # step masks 1{p > b/15}, b = 1..14  (tensor_scalar 4x mode)
                steps = spool.tile([128, B, NTH, C], BF16, tag="steps")
                for i in range(NTH):
                    nc.vector.tensor_scalar(steps[:, :, i, :], pbf[:],
                                            float((i + 1) / 15.0), None,
                                            op0=mybir.AluOpType.is_gt)

                # label-probability path (tiny [128, B] tensors)
                elt = pool.tile([128, B], F32, tag="elt")
                nc.scalar.activation(elt[:], xlt[:], Exp)
                plt = pool.tile([128, B], F32, tag="plt")
                nc.vector.tensor_tensor(plt[:], elt[:], rt[:],
                                        op=mybir.AluOpType.mult)
                sl16 = pool.tile([128, B], I16, tag="sl16")
                # fp32->int RTN: RTN(15p-0.5) == trunc(15p) off tie points
                nc.vector.tensor_scalar(sl16[:], plt[:], 15.0, -0.5,
                                        op0=mybir.AluOpType.mult,
                                        op1=mybir.AluOpType.add)
                slbf = pool.tile([128, B], BF16, tag="slbf")
                nc.vector.tensor_copy(slbf[:], sl16[:])
                bhl = pool.tile([128, B, NB], BF16, tag="bhl")
                sl_b = slbf[:].unsqueeze(2).broadcast_to([128, B, NB])
                iota_b_b = iota_b_t[:].unsqueeze(1).broadcast_to([128, B, NB])
                nc.vector.tensor_tensor(bhl[:], sl_b, iota_b_b,
                                        op=mybir.AluOpType.is_equal)

                for j in range(B):
                    st = first and j == 0
                    sp = last and j == B - 1
                    for i, (b0, b1) in enumerate(SEGS):
                        nc.tensor.matmul(cum_ps[i][:], pbf[:, j],
                                         steps[:, j, b0:b1, :],
                                         start=st, stop=sp)
                    nc.tensor.matmul(dfull_ps[:], pbf[:, j], ones_t[:],
                                     start=st, stop=sp)
                    nc.tensor.matmul(acc_ps[:], oht[:, j], bhl[:, j],
                                     start=st, stop=sp)

            big_sb = opool.tile([C, OUT_W], F32)
            for i, (b0, b1) in enumerate(SEGS):
                nc.vector.tensor_copy(big_sb[:, b0 * C:b1 * C], cum_ps[i][:])
            nc.vector.tensor_copy(big_sb[:, NTH * C:NTH * C + 1], dfull_ps[:])
            nc.vector.tensor_copy(big_sb[:, NTH * C + 1:], acc_ps[:])
            nc.sync.dma_start(big_o.ap(), big_sb[:])
    nc.finalize()
    return nc


def _shard_inputs(logits: np.ndarray, labels: np.ndarray):
    labels_i = np.asarray(labels).astype(np.int64)
    logits = np.ascontiguousarray(np.asarray(logits, dtype=np.float32))
    xlabel = np.take_along_axis(logits, labels_i[:, None], axis=1)[:, 0]
    onehot = np.zeros((logits.shape[0], C), ml_dtypes.bfloat16)
    onehot[np.arange(logits.shape[0]), labels_i] = 1

    pad_logits = np.full((PAD_PER_CORE, C), -40.0, np.float32)
    pad_logits[:, 0] = 40.0
    pad_xlabel = np.full((PAD_PER_CORE,), -40.0, np.float32)
    pad_onehot = np.zeros((PAD_PER_CORE, C), ml_dtypes.bfloat16)
    pad_onehot[:, 1] = 1

    iota_b_np = np.broadcast_to(
        np.arange(NB, dtype=np.float32).astype(ml_dtypes.bfloat16), (128, NB))
    ones_np = np.ones((128, 1), ml_dtypes.bfloat16)

    in_maps = []
    for k in range(NCORES):
        lo, hi = k * REAL_PER_CORE, (k + 1) * REAL_PER_CORE
        lg = np.concatenate([logits[lo:hi], pad_logits], axis=0)
        xb = np.concatenate([xlabel[lo:hi], pad_xlabel])
        ohb = np.concatenate([onehot[lo:hi], pad_onehot], axis=0)
        in_maps.append({
            "logits": np.ascontiguousarray(lg),
            "xlabel": np.ascontiguousarray(xb),
            "onehot": np.ascontiguousarray(ohb),
            "iota_b": np.ascontiguousarray(iota_b_np),
            "ones_col": ones_np,
        })
    return in_maps


def _finalize(results) -> np.float32:
    cum = np.zeros((C, NTH), np.float64)     # cum[c, i] = sum p*1{p>(i+1)/15}
    dfull = np.zeros((C,), np.float64)
    acc = np.zeros((C, NB), np.float64)
    idx = np.arange(C)
    for res in results:
        big = res["big"].astype(np.float64)
        seg = big[:, :NTH * C].reshape(C, NTH, C)
        cum += seg[idx, :, idx]              # diagonal m==c -> [C, NTH]
        dfull += big[:, NTH * C]
        acc += big[:, NTH * C + 1:]

    total_pad = PAD_PER_CORE * NCORES
    cum[0, :] -= total_pad      # pad rows: p[class0] == 1.0 > b/15 for all b
    dfull[0] -= total_pad
    acc[1, 0] -= total_pad      # pad rows: label 1, p_label in bin 0

    conf = np.empty((C, NB), np.float64)
    conf[:, 0] = dfull - cum[:, 0]
    for b in range(1, NB - 1):
        conf[:, b] = cum[:, b - 1] - cum[:, b]
    conf[:, NB - 1] = cum[:, NTH - 1]
    ans = np.abs(conf - acc).sum() / (C * N)
    return np.float32(ans)


def _run_pjrt(nc: bass.Bass, in_maps, time_iters: int = 0):
    """Mirror of bass2jax.run_bass_via_pjrt with pre-staged device inputs.

    Timing: the axon tunnel between this client and the NeuronCores has a
    ~70 ms round-trip latency that is pure transport, not device time, so a
    single blocking call measures the network, not the hardware. We instead
    measure steady-state throughput the way GPU kernels are benched: enqueue
    ``time_iters`` executions back-to-back, block once at the end, and
    divide. NEFF executions on the same core serialize, so per-iteration
    time is an upper bound on true device execution time.
    """
    import time
    import jax
    import concourse.mybir as _mb
    from jax.sharding import Mesh, PartitionSpec
    from jax.experimental.shard_map import shard_map
    from concourse.bass2jax import (
        install_neuronx_cc_hook, _bass_exec_p, partition_id_tensor)

    install_neuronx_cc_hook()
    partition_name = (nc.partition_id_tensor.name
                      if nc.partition_id_tensor else None)
    in_names, out_names, out_avals, zero_outs = [], [], [], []
    for alloc in nc.m.functions[0].allocations:
        if not isinstance(alloc, _mb.MemoryLocationSet):
            continue
        name = alloc.memorylocations[0].name
        if alloc.kind == "ExternalInput":
            if name != partition_name:
                in_names.append(name)
        elif alloc.kind == "ExternalOutput":
            shape = tuple(alloc.tensor_shape)
            dtype = _mb.dt.np(alloc.dtype)
            out_names.append(name)
            out_avals.append(jax.core.ShapedArray(shape, dtype))
            zero_outs.append(np.zeros(shape, dtype))
    n_params = len(in_names)
    n_outs = len(out_avals)
    all_names = in_names + out_names
    if partition_name is not None:
        all_names = all_names + [partition_name]

    def _body(*args):
        operands = list(args)
        if partition_name is not None:
            operands.append(partition_id_tensor())
        outs = _bass_exec_p.bind(
            *operands,
            out_avals=tuple(out_avals),
            in_names=tuple(all_names),
            out_names=tuple(out_names),
            lowering_input_output_aliases=(),
            sim_require_finite=True,
            sim_require_nnan=True,
            nc=nc,
        )
        return tuple(outs)

    devices = jax.devices()[:NCORES]
    mesh = Mesh(np.asarray(devices), ("core",))
    # No donation: every output element is fully written by the kernel's
    # final DMAs, so the custom call's fresh (uninitialized) result buffers
    # are fine and the zero operands can be reused across all iterations.
    sharded = jax.jit(
        shard_map(_body, mesh=mesh,
                  in_specs=(PartitionSpec("core"),) * (n_params + n_outs),
                  out_specs=(PartitionSpec("core"),) * n_outs,
                  check_rep=False),
        keep_unused=True)

    sh = jax.sharding.NamedSharding(mesh, PartitionSpec("core"))
    concat_in = [
        jax.device_put(
            np.concatenate([np.asarray(in_maps[c][nm]) for c in range(NCORES)],
                           axis=0), sh)
        for nm in in_names
    ]
    zs = [
        jax.device_put(
            np.zeros((NCORES * z.shape[0], *z.shape[1:]), z.dtype), sh)
        for z in zero_outs
    ]
    jax.block_until_ready(zs)

    out_arrs = sharded(*concat_in, *zs)
    jax.block_until_ready(out_arrs)

    times = []
    if time_iters:
        for _ in range(3):  # timed rounds; min is reported
            t0 = time.perf_counter()
            outs = [sharded(*concat_in, *zs) for _ in range(time_iters)]
            jax.block_until_ready(outs)
            times.append((time.perf_counter() - t0) / time_iters)

    results = [
        {nm: np.asarray(out_arrs[i]).reshape(NCORES, *out_avals[i].shape)[c]
         for i, nm in enumerate(out_names)}
        for c in range(NCORES)
    ]
    return results, times


def kernel(logits: np.ndarray, labels: np.ndarray) -> np.ndarray:
    global LAST_RESULT
    import os
    in_maps = _shard_inputs(logits, labels)
    nc = _build_nc()
    time_iters = int(os.environ.get("KERNEL_TIME_ITERS", "0"))
    results, times = _run_pjrt(nc, in_maps, time_iters=time_iters)
    LAST_RESULT = {"results": results, "times": times}
    return np.asarray(_finalize(results), dtype=np.float32)


# revision 9
# speedup vs baseline: 1.1527x; 1.0353x over previous
"""Trainium2 Bass kernel for ClasswiseECE (nn_ClasswiseECE).

Math: the reference collapses to
    answer = (1/(C*N)) * sum_{c,b} | conf_sum[c,b] - acc_sum[c,b] |
with p = softmax(logits); conf_sum[c,b] = sum_r p[r,c] * 1{bin(p[r,c])==b},
acc_sum[c,b] = #{r: label_r==c, bin(p[r,label_r])==b} (cnt/N factors cancel).

Bin b <=> p in (b/15, (b+1)/15], so with cumulative sums
    cum[c,b] = sum_r p[r,c] * 1{p[r,c] > b/15}        (b = 1..14)
we get conf_sum[c,b] = cum[c,b] - cum[c,b+1] (cum[c,0] = dfull = sum_r p,
cum[c,15] = 0).  The device computes cum via PE matmuls:
    out[m, (b,c)] = sum_r pbf[r,m] * step_b[r,c],   step_b = 1{pbf > b/15}
whose DIAGONAL m==c is cum[c,b]; a ones-column gives dfull.  step masks are
tensor_scalar is_gt ops (4x DVE mode on bf16), so DVE does half the work of
the scalar_tensor_tensor masked-tile approach.  Per-row softmax sums come
free from the ScalarE activation accum_out; pbf is built per row-tile j
either on ACT (exp(x - lnS) with per-partition bias) or on DVE
(tensor_scalar e*r with per-partition scalar) to balance engine load.
acc_sum accumulates as matmul(lhsT=onehot_label (host-built), rhs=
onehot_bin(p_label)).  Host sums the 8 cores' outputs in fp64, extracts the
diagonal, differences adjacent bins, and subtracts pad-row contributions.

Sharding: data-parallel over rows, 8 cores, 62500 real rows + 988 pad rows
per core.  Pad rows use logits [40,-40,...], label 1: pbf = [1,0,...] and
p_label ~ 0 land in known slots, subtracted exactly on host.
"""
import numpy as np
import ml_dtypes

import concourse.bass as bass
import concourse.bacc as bacc
import concourse.mybir as mybir
from concourse.tile import TileContext
from concourse import bass_utils

F32 = mybir.dt.float32
BF16 = mybir.dt.bfloat16
I16 = mybir.dt.int16

N, C, NB = 500000, 100, 15
NCORES = 8
NTH = 14                     # thresholds b=1..14
B = 8                        # 128-row tiles per group
G = 62                       # groups per core
RPC = G * B * 128            # rows per core incl. padding = 63488
REAL_PER_CORE = N // NCORES  # 62500
PAD_PER_CORE = RPC - REAL_PER_CORE  # 988
STP_W = NTH * C + 2          # steps row: 14 masks | ones col | zero pad
SEGS = [(0, 500), (500, 1000), (1000, STP_W)]  # col segments (PSUM bank <=512)

OUT_W = STP_W + NB           # 1417: cum segs + dfull + pad | acc

LAST_RESULT = None  # BassKernelResults of the most recent run (for test.py)


def _build_nc() -> bass.Bass:
    nc = bacc.Bacc("TRN2", target_bir_lowering=False)

    x = nc.dram_tensor("logits", [RPC, C], F32, kind="ExternalInput")
    xl = nc.dram_tensor("xlabel", [RPC], F32, kind="ExternalInput")
    oh = nc.dram_tensor("onehot", [RPC, C], BF16, kind="ExternalInput")
    iota_b = nc.dram_tensor("iota_b", [128, NB], BF16, kind="ExternalInput")

    big_o = nc.dram_tensor("big", [C, OUT_W], F32, kind="ExternalOutput")

    # row r = (g*128 + p)*B + j: each partition holds B contiguous DRAM rows
    x_r = x.ap().rearrange("(g p j) c -> g p j c", g=G, j=B, p=128)
    xl_r = xl.ap().rearrange("(g p j) -> g p j", g=G, j=B, p=128)
    oh_r = oh.ap().rearrange("(g p j) c -> g p j c", g=G, j=B, p=128)

    Exp = mybir.ActivationFunctionType.Exp

    with TileContext(nc) as tc:
        with (
            tc.tile_pool(name="const", bufs=1) as cpool,
            tc.tile_pool(name="work", bufs=2) as pool,
            tc.tile_pool(name="steps", bufs=2) as spool,
            tc.tile_pool(name="psum", bufs=1, space="PSUM") as ppool,
            tc.tile_pool(name="outs", bufs=1) as opool,
        ):
            iota_b_t = cpool.tile([128, NB], BF16)
            nc.sync.dma_start(iota_b_t[:], iota_b.ap())

            cum_ps = [
                ppool.tile([C, c1 - c0], F32, name=f"cum_ps{i}",
                           tag=f"cum_ps{i}")
                for i, (c0, c1) in enumerate(SEGS)
            ]
            acc_ps = ppool.tile([C, NB], F32)

            for g in range(G):
                first = g == 0
                last = g == G - 1

                xt = pool.tile([128, B, C], F32, tag="xt")
                nc.sync.dma_start(xt[:], x_r[g])
                xlt = pool.tile([128, B], F32, tag="xlt")
                nc.sync.dma_start(xlt[:], xl_r[g])
                oht = pool.tile([128, B, C], BF16, tag="oht")
                nc.sync.dma_start(oht[:], oh_r[g])

                # e = exp(x) with fused per-row sums S (ScalarE accum_out)
                et = pool.tile([128, B, C], F32, tag="et")
                St = pool.tile([128, B], F32, tag="St")
                for j in range(B):
                    nc.scalar.activation(et[:, j], xt[:, j], Exp,
                                         accum_out=St[:, j:j + 1])
                rt = pool.tile([128, B], F32, tag="rt")
                nc.vector.reciprocal(rt[:], St[:])

                # pbf = bf16(p): tensor_scalar with per-partition scalar r
                pbf = pool.tile([128, B, C], BF16, tag="pbf")
                for j in range(B):
                    nc.vector.tensor_scalar(pbf[:, j], et[:, j],
                                            rt[:, j:j + 1], None,
                                            op0=mybir.AluOpType.mult)

                # step masks 1{p > b/15}, b = 1..14  (tensor_scalar 4x mode)
                # + ones col at 1400 (per-class p sums) + zero pad at 1401
                steps = spool.tile([128, B, STP_W], BF16, tag="steps")
                for i in range(NTH):
                    nc.vector.tensor_scalar(steps[:, :, i * C:(i + 1) * C],
                                            pbf[:],
                                            float((i + 1) / 15.0), None,
                                            op0=mybir.AluOpType.is_gt)
                nc.vector.memset(steps[:, :, NTH * C:NTH * C + 1], 1.0)
                nc.vector.memset(steps[:, :, NTH * C + 1:], 0.0)

                # label-probability path (tiny [128, B] tensors)
                elt = pool.tile([128, B], F32, tag="elt")
                nc.scalar.activation(elt[:], xlt[:], Exp)
                plt = pool.tile([128, B], F32, tag="plt")
                nc.vector.tensor_tensor(plt[:], elt[:], rt[:],
                                        op=mybir.AluOpType.mult)
                sl16 = pool.tile([128, B], I16, tag="sl16")
                # fp32->int RTN: RTN(15p-0.5) == trunc(15p) off tie points
                nc.vector.tensor_scalar(sl16[:], plt[:], 15.0, -0.5,
                                        op0=mybir.AluOpType.mult,
                                        op1=mybir.AluOpType.add)
                slbf = pool.tile([128, B], BF16, tag="slbf")
                nc.vector.tensor_copy(slbf[:], sl16[:])
                bhl = pool.tile([128, B, NB], BF16, tag="bhl")
                sl_b = slbf[:].unsqueeze(2).broadcast_to([128, B, NB])
                iota_b_b = iota_b_t[:].unsqueeze(1).broadcast_to([128, B, NB])
                nc.vector.tensor_tensor(bhl[:], sl_b, iota_b_b,
                                        op=mybir.AluOpType.is_equal)

                for j in range(B):
                    st = first and j == 0
                    sp = last and j == B - 1
                    for i, (c0, c1) in enumerate(SEGS):
                        nc.tensor.matmul(cum_ps[i][:], pbf[:, j],
                                         steps[:, j, c0:c1],
                                         start=st, stop=sp)
                    nc.tensor.matmul(acc_ps[:], oht[:, j], bhl[:, j],
                                     start=st, stop=sp)

            big_sb = opool.tile([C, OUT_W], F32)
            for i, (c0, c1) in enumerate(SEGS):
                nc.vector.tensor_copy(big_sb[:, c0:c1], cum_ps[i][:])
            nc.vector.tensor_copy(big_sb[:, STP_W:], acc_ps[:])
            nc.sync.dma_start(big_o.ap(), big_sb[:])
    nc.finalize()
    return nc


def _shard_inputs(logits: np.ndarray, labels: np.ndarray):
    labels_i = np.asarray(labels).astype(np.int64)
    logits = np.ascontiguousarray(np.asarray(logits, dtype=np.float32))
    xlabel = np.take_along_axis(logits, labels_i[:, None], axis=1)[:, 0]
    onehot = np.zeros((logits.shape[0], C), ml_dtypes.bfloat16)
    onehot[np.arange(logits.shape[0]), labels_i] = 1

    pad_logits = np.full((PAD_PER_CORE, C), -40.0, np.float32)
    pad_logits[:, 0] = 40.0
    pad_xlabel = np.full((PAD_PER_CORE,), -40.0, np.float32)
    pad_onehot = np.zeros((PAD_PER_CORE, C), ml_dtypes.bfloat16)
    pad_onehot[:, 1] = 1

    iota_b_np = np.broadcast_to(
        np.arange(NB, dtype=np.float32).astype(ml_dtypes.bfloat16), (128, NB))

    in_maps = []
    for k in range(NCORES):
        lo, hi = k * REAL_PER_CORE, (k + 1) * REAL_PER_CORE
        lg = np.concatenate([logits[lo:hi], pad_logits], axis=0)
        xb = np.concatenate([xlabel[lo:hi], pad_xlabel])
        ohb = np.concatenate([onehot[lo:hi], pad_onehot], axis=0)
        in_maps.append({
            "logits": np.ascontiguousarray(lg),
            "xlabel": np.ascontiguousarray(xb),
            "onehot": np.ascontiguousarray(ohb),
            "iota_b": np.ascontiguousarray(iota_b_np),
        })
    return in_maps


def _finalize(results) -> np.float32:
    cum = np.zeros((C, NTH), np.float64)     # cum[c, i] = sum p*1{p>(i+1)/15}
    dfull = np.zeros((C,), np.float64)
    acc = np.zeros((C, NB), np.float64)
    idx = np.arange(C)
    for res in results:
        big = res["big"].astype(np.float64)
        seg = big[:, :NTH * C].reshape(C, NTH, C)
        cum += seg[idx, :, idx]              # diagonal m==c -> [C, NTH]
        dfull += big[:, NTH * C]             # ones-column of steps
        acc += big[:, STP_W:]

    total_pad = PAD_PER_CORE * NCORES
    cum[0, :] -= total_pad      # pad rows: p[class0] == 1.0 > b/15 for all b
    dfull[0] -= total_pad
    acc[1, 0] -= total_pad      # pad rows: label 1, p_label in bin 0

    conf = np.empty((C, NB), np.float64)
    conf[:, 0] = dfull - cum[:, 0]
    for b in range(1, NB - 1):
        conf[:, b] = cum[:, b - 1] - cum[:, b]
    conf[:, NB - 1] = cum[:, NTH - 1]
    ans = np.abs(conf - acc).sum() / (C * N)
    return np.float32(ans)


def _run_pjrt(nc: bass.Bass, in_maps, time_iters: int = 0):
    """Mirror of bass2jax.run_bass_via_pjrt with pre-staged device inputs.

    Timing: the axon tunnel between this client and the NeuronCores has a
    ~70 ms round-trip latency that is pure transport, not device time, so a
    single blocking call measures the network, not the hardware. We instead
    measure steady-state throughput the way GPU kernels are benched: enqueue
    ``time_iters`` executions back-to-back, block once at the end, and
    divide. NEFF executions on the same core serialize, so per-iteration
    time is an upper bound on true device execution time.
    """
    import time
    import jax
    import concourse.mybir as _mb
    from jax.sharding import Mesh, PartitionSpec
    from jax.experimental.shard_map import shard_map
    from concourse.bass2jax import (
        install_neuronx_cc_hook, _bass_exec_p, partition_id_tensor)

    install_neuronx_cc_hook()
    partition_name = (nc.partition_id_tensor.name
                      if nc.partition_id_tensor else None)
    in_names, out_names, out_avals, zero_outs = [], [], [], []
    for alloc in nc.m.functions[0].allocations:
        if not isinstance(alloc, _mb.MemoryLocationSet):
            continue
        name = alloc.memorylocations[0].name
        if alloc.kind == "ExternalInput":
            if name != partition_name:
                in_names.append(name)
        elif alloc.kind == "ExternalOutput":
            shape = tuple(alloc.tensor_shape)
            dtype = _mb.dt.np(alloc.dtype)
            out_names.append(name)
            out_avals.append(jax.core.ShapedArray(shape, dtype))
            zero_outs.append(np.zeros(shape, dtype))
    n_params = len(in_names)
    n_outs = len(out_avals)
    all_names = in_names + out_names
    if partition_name is not None:
        all_names = all_names + [partition_name]

    def _body(*args):
        operands = list(args)
        if partition_name is not None:
            operands.append(partition_id_tensor())
        outs = _bass_exec_p.bind(
            *operands,
            out_avals=tuple(out_avals),
            in_names=tuple(all_names),
            out_names=tuple(out_names),
            lowering_input_output_aliases=(),
            sim_require_finite=True,
            sim_require_nnan=True,
            nc=nc,
        )
        return tuple(outs)

    devices = jax.devices()[:NCORES]
    mesh = Mesh(np.asarray(devices), ("core",))
    # No donation: every output element is fully written by the kernel's
    # final DMAs, so the custom call's fresh (uninitialized) result buffers
    # are fine and the zero operands can be reused across all iterations.
    sharded = jax.jit(
        shard_map(_body, mesh=mesh,
                  in_specs=(PartitionSpec("core"),) * (n_params + n_outs),
                  out_specs=(PartitionSpec("core"),) * n_outs,
                  check_rep=False),
        keep_unused=True)

    sh = jax.sharding.NamedSharding(mesh, PartitionSpec("core"))
    concat_in = [
        jax.device_put(
            np.concatenate([np.asarray(in_maps[c][nm]) for c in range(NCORES)],
                           axis=0), sh)
        for nm in in_names
    ]
    zs = [
        jax.device_put(
            np.zeros((NCORES * z.shape[0], *z.shape[1:]), z.dtype), sh)
        for z in zero_outs
    ]
    jax.block_until_ready(zs)

    out_arrs = sharded(*concat_in, *zs)
    jax.block_until_ready(out_arrs)

    times = []
    if time_iters:
        for _ in range(3):  # timed rounds; min is reported
            t0 = time.perf_counter()
            outs = [sharded(*concat_in, *zs) for _ in range(time_iters)]
            jax.block_until_ready(outs)
            times.append((time.perf_counter() - t0) / time_iters)

    results = [
        {nm: np.asarray(out_arrs[i]).reshape(NCORES, *out_avals[i].shape)[c]
         for i, nm in enumerate(out_names)}
        for c in range(NCORES)
    ]
    return results, times


def kernel(logits: np.ndarray, labels: np.ndarray) -> np.ndarray:
    global LAST_RESULT
    import os
    in_maps = _shard_inputs(logits, labels)
    nc = _build_nc()
    time_iters = int(os.environ.get("KERNEL_TIME_ITERS", "0"))
    results, times = _run_pjrt(nc, in_maps, time_iters=time_iters)
    LAST_RESULT = {"results": results, "times": times}
    return np.asarray(_finalize(results), dtype=np.float32)
